# revision 6
# baseline (speedup 1.0000x reference)
"""Causal self-attention on 8 Trainium2 NeuronCores.

Problem: y = CausalSelfAttention(x) with B=2, T=2048, C=1024, NH=16, HD=64.
  qkv = x @ w_qkv ; per-head causal softmax attention ; out = y @ w_proj

Sharding (core c = 4*b + g): data-parallel over batch b (2-way), tensor-
parallel over heads (4-way head-groups g, column-split w_qkv / row-split
w_proj).  Each core computes a full [T, C] partial projection output; the
host sums the 4 partials per batch (the "all-reduce").

Device-side layout trick: the host passes x[b] TRANSPOSED ([C, T]) so the
contraction dim (C) is already on SBUF partitions — no on-device transposes
anywhere.  Attention is computed in the transposed orientation
(S^T = K^T.T @ Q^T with T_k on partitions), so the softmax reduction over
keys is a matmul with a ones vector and A@V needs no transposes either.

All matmuls run in float32r (fp22 mantissa, 1 PE cycle/row at N>=256).
"""

import numpy as np

import concourse.bass as bass
import concourse.tile as tile
import concourse.mybir as mybir
from concourse import bacc
from concourse.bass_utils import run_bass_kernel_spmd

F32 = mybir.dt.float32
F32R = mybir.dt.float32r

B, T, C = 2, 2048, 1024
NH, HD = 16, 64
NCORES = 8
HPC = 4                 # heads per core
WQKV_SL = HPC * HD      # 256 w_qkv columns per section per core
NT = T // 128           # 16 T-chunks of 128
NCC = C // 128          # 8 C-chunks of 128
NG = T // 512           # 4 query groups of 512
MASK_NEG = -1.0e9
DEBUG_DUMPS = False


DBG = {}


def _attention_body(tc):
    nc = tc.nc
    xt_d = nc.dram_tensor("xt", [C, T], F32, kind="ExternalInput")
    wq_d = nc.dram_tensor("wq", [C, WQKV_SL], F32, kind="ExternalInput")
    wk_d = nc.dram_tensor("wk", [C, WQKV_SL], F32, kind="ExternalInput")
    wv_d = nc.dram_tensor("wv", [C, WQKV_SL], F32, kind="ExternalInput")
    wp_d = nc.dram_tensor("wp", [WQKV_SL, C], F32, kind="ExternalInput")
    out_d = nc.dram_tensor("out", [T, C], F32, kind="ExternalOutput")

    Exp = mybir.ActivationFunctionType.Exp

    with (
        tc.tile_pool(name="big", bufs=1) as big,
        tc.tile_pool(name="wts", bufs=1) as wts,
        tc.tile_pool(name="pt", bufs=3) as ptp,
        tc.tile_pool(name="outp", bufs=3) as outp,
        tc.tile_pool(name="norm", bufs=2) as normp,
        tc.tile_pool(name="ps_s", bufs=2, space="PSUM") as ps_s,
        tc.tile_pool(name="ps_y", bufs=2, space="PSUM") as ps_y,
        tc.tile_pool(name="ps_d", bufs=2, space="PSUM") as ps_d,
    ):
        # ---- constants -------------------------------------------------
        mask_sb = wts.tile([128, 128], F32, tag="mask")
        nc.gpsimd.memset(mask_sb[:, :], 0.0)
        # keep 0 where col >= row (query >= key), else MASK_NEG
        nc.gpsimd.affine_select(
            out=mask_sb[:, :], in_=mask_sb[:, :],
            compare_op=mybir.AluOpType.is_ge,
            fill=MASK_NEG, base=0,
            pattern=[[1, 128]], channel_multiplier=-1,
        )
        # ---- input loads ----------------------------------------------
        xt_sb = big.tile([128, NCC, T], F32R, tag="xt")
        for cc in range(NCC):
            nc.sync.dma_start(
                out=xt_sb[:, cc, :],
                in_=xt_d.ap()[128 * cc:128 * (cc + 1), :].bitcast(F32R),
            )
        wq_sb = wts.tile([128, NCC, WQKV_SL], F32R, tag="wq")
        wk_sb = wts.tile([128, NCC, WQKV_SL], F32R, tag="wk")
        wv_sb = wts.tile([128, NCC, WQKV_SL], F32R, tag="wv")
        for w_sb, w_d in ((wq_sb, wq_d), (wk_sb, wk_d), (wv_sb, wv_d)):
            nc.sync.dma_start(
                out=w_sb[:, :, :],
                in_=w_d.ap().rearrange("(cc p) n -> p cc n", p=128).bitcast(F32R),
            )
        wp_sb = wts.tile([128, 2, C], F32R, tag="wp")
        nc.sync.dma_start(
            out=wp_sb[:, :, :],
            in_=wp_d.ap().rearrange("(k p) n -> p k n", p=128).bitcast(F32R),
        )

        # ---- QKV ------------------------------------------------------
        # Q^T, K^T: [256 ch, T] as [128, 2, T] tiles (orientation 2)
        qt = big.tile([128, 2, T], F32R, tag="qt")
        kt = big.tile([128, 2, T], F32R, tag="kt")
        ypools = (ps_y, ps_d)
        pi = 0
        for w_sb, dst in ((wq_sb, qt), (wk_sb, kt)):
            for k in range(2):
                for tg in range(NG):
                    ps = ypools[pi % 2].tile([128, 512], F32, tag="acc")
                    pi += 1
                    for cc in range(NCC):
                        nc.tensor.matmul(
                            ps[:, :],
                            lhsT=w_sb[:, cc, 128 * k:128 * (k + 1)],
                            rhs=xt_sb[:, cc, 512 * tg:512 * (tg + 1)],
                            start=(cc == 0), stop=(cc == NCC - 1),
                        )
                    nc.vector.tensor_copy(
                        out=dst[:, k, 512 * tg:512 * (tg + 1)], in_=ps[:, :]
                    )
        # V: natural [T, 4 heads, 64+1] with an appended ones column per head
        # (the ones column makes the A@V matmul also produce the softmax
        # denominator in psum row 64)
        v_sb = big.tile([128, NT, HPC, HD + 1], F32R, tag="v")
        ones_sb = wts.tile([128, NT * HPC], F32, tag="ones")
        nc.vector.memset(ones_sb[:, :], 1.0)
        nc.vector.tensor_copy(
            out=v_sb[:, :, :, HD:HD + 1],
            in_=ones_sb[:, :].rearrange("p (a b c) -> p a b c", a=NT, b=HPC),
        )
        for ti in range(NT):
            ps = ypools[pi % 2].tile([128, WQKV_SL], F32, tag="acc")
            pi += 1
            for cc in range(NCC):
                nc.tensor.matmul(
                    ps[:, :],
                    lhsT=xt_sb[:, cc, 128 * ti:128 * (ti + 1)],
                    rhs=wv_sb[:, cc, :],
                    start=(cc == 0), stop=(cc == NCC - 1),
                )
            nc.vector.tensor_copy(
                out=v_sb[:, ti, :, 0:HD],
                in_=ps[:, :].rearrange("p (h d) -> p h d", h=HPC),
            )

        # ---- attention -------------------------------------------------
        # Y^T: [256 ch, T] as [128, 2, T]; pair p holds heads (2p, 2p+1)
        yt = big.tile([128, 2, T], F32R, tag="yt")
        DBG.update(qt=qt.name, kt=kt.name, v=v_sb.name, yt=yt.name, xt=xt_sb.name)
        for pair in range(2):
            for g in range(NG):
                nch = 4 * g + 4
                ya_ps = ps_y.tile([128, 512], F32, tag="acc")
                yb_ps = ps_d.tile([128, 512], F32, tag="acc")
                for j in range(nch):
                    # S^T for both heads of the pair, row-packed on the PE
                    s_ps = ps_s.tile([128, 1024], F32, tag="s")
                    pt = ptp.tile([128, 1024], F32R, tag="pt")
                    for rowb, colb in ((0, 0), (64, 512)):
                        nc.tensor.matmul(
                            s_ps[:, colb:colb + 512],
                            lhsT=kt[rowb:rowb + 64, pair, 128 * j:128 * (j + 1)],
                            rhs=qt[rowb:rowb + 64, pair, 512 * g:512 * (g + 1)],
                            start=True, stop=True,
                            tile_position=(rowb, 0),
                        )
                    if j >= 4 * g:  # diagonal chunk: mask the boundary subtile
                        r = j - 4 * g
                        for colb in (0, 512):
                            cs = colb + 128 * r
                            nc.vector.tensor_add(
                                s_ps[:, cs:cs + 128], s_ps[:, cs:cs + 128],
                                mask_sb[:, :],
                            )
                    nc.scalar.activation(
                        out=pt[:, :], in_=s_ps[:, :], func=Exp, scale=1.0 / 8.0,
                    )
                    # A@V per head (M=65: the ones column lands the softmax
                    # denominator in psum row 64), accumulated over j
                    c0 = 128 * (j - 4 * g) if j >= 4 * g else 0
                    st, sp = (j == 0), (j == nch - 1)
                    nc.tensor.matmul(
                        ya_ps[0:65, c0:512],
                        lhsT=v_sb[:, j, 2 * pair, :],
                        rhs=pt[:, c0:512],
                        start=st, stop=sp,
                    )
                    nc.tensor.matmul(
                        yb_ps[0:65, c0:512],
                        lhsT=v_sb[:, j, 2 * pair + 1, :],
                        rhs=pt[:, 512 + c0:1024],
                        start=st, stop=sp,
                    )
                # normalize: yt rows 0-63 = yA/sA, rows 64-127 = yB/sB
                # NB: partition_broadcast reads the tile's physical partition
                # 0 (it ignores the AP base partition), so each reciprocal
                # gets its own tile at partition 0.
                recipa_sb = normp.tile([1, 512], F32, tag="recipa")
                recipb_sb = normp.tile([1, 512], F32, tag="recipb")
                bcasta_sb = normp.tile([64, 512], F32, tag="bcasta")
                bcastb_sb = normp.tile([64, 512], F32, tag="bcastb")
                nc.vector.reciprocal(out=recipa_sb[0:1, :], in_=ya_ps[64:65, :])
                nc.vector.reciprocal(out=recipb_sb[0:1, :], in_=yb_ps[64:65, :])
                nc.gpsimd.partition_broadcast(bcasta_sb[:, :], recipa_sb[0:1, :])
                nc.gpsimd.partition_broadcast(bcastb_sb[:, :], recipb_sb[0:1, :])
                gsl = slice(512 * g, 512 * (g + 1))
                nc.vector.tensor_mul(
                    yt[0:64, pair, gsl], ya_ps[0:64, :], bcasta_sb[:, :]
                )
                nc.vector.tensor_mul(
                    yt[64:128, pair, gsl], yb_ps[0:64, :], bcastb_sb[:, :]
                )

        if DEBUG_DUMPS:
            for nm, tl in (("d_qt", qt), ("d_kt", kt), ("d_yt", yt)):
                dd = nc.dram_tensor(nm, [128, 2 * T], F32, kind="ExternalOutput")
                nc.sync.dma_start(out=dd.ap(),
                                  in_=tl[:, :, :].bitcast(F32))
            dv = nc.dram_tensor("d_v", [128, NT * HPC * (HD + 1)], F32,
                                kind="ExternalOutput")
            nc.sync.dma_start(out=dv.ap(), in_=v_sb[:, :, :, :].bitcast(F32))

        # ---- projection ------------------------------------------------
        for ti in range(NT):
            for n2 in range(2):
                ps = ypools[pi % 2].tile([128, 512], F32, tag="acc")
                pi += 1
                for k in range(2):
                    nc.tensor.matmul(
                        ps[:, :],
                        lhsT=yt[:, k, 128 * ti:128 * (ti + 1)],
                        rhs=wp_sb[:, k, 512 * n2:512 * (n2 + 1)],
                        start=(k == 0), stop=(k == 1),
                    )
                o_sb = outp.tile([128, 512], F32, tag="o")
                nc.vector.tensor_copy(out=o_sb[:, :], in_=ps[:, :])
                nc.sync.dma_start(
                    out=out_d.ap()[128 * ti:128 * (ti + 1),
                                   512 * n2:512 * (n2 + 1)],
                    in_=o_sb[:, :],
                )


_NC_CACHE = None


def build_nc():
    global _NC_CACHE
    if _NC_CACHE is not None:
        return _NC_CACHE
    nc = bacc.Bacc("TRN2", target_bir_lowering=False, debug=False,
                   num_devices=NCORES)
    with tile.TileContext(nc) as tc:
        _attention_body(tc)
    nc.compile()
    _NC_CACHE = nc
    return nc


def make_in_maps(x, w_qkv, w_proj):
    x = np.asarray(x, dtype=np.float32)
    w_qkv = np.asarray(w_qkv, dtype=np.float32)
    w_proj = np.asarray(w_proj, dtype=np.float32)
    in_maps = []
    for c in range(NCORES):
        b, g = divmod(c, NCORES // B)
        sl = slice(WQKV_SL * g, WQKV_SL * (g + 1))
        in_maps.append({
            "xt": np.ascontiguousarray(x[b].T),
            "wq": np.ascontiguousarray(w_qkv[:, sl]),
            "wk": np.ascontiguousarray(w_qkv[:, C:][:, sl]),
            "wv": np.ascontiguousarray(w_qkv[:, 2 * C:][:, sl]),
            "wp": np.ascontiguousarray(w_proj[sl, :]),
        })
    return in_maps


def kernel(x, w_qkv, w_proj):
    nc = build_nc()
    in_maps = make_in_maps(x, w_qkv, w_proj)
    res = run_bass_kernel_spmd(nc, in_maps, core_ids=list(range(NCORES)))
    out = np.empty((B, T, C), dtype=np.float32)
    npc = NCORES // B
    for b in range(B):
        acc = res.results[npc * b]["out"].astype(np.float32)
        for g in range(1, npc):
            acc = acc + res.results[npc * b + g]["out"]
        out[b] = acc
    return out


# revision 11
# speedup vs baseline: 1.4232x; 1.4232x over previous
"""Causal self-attention on 8 Trainium2 NeuronCores.

Problem: y = CausalSelfAttention(x) with B=2, T=2048, C=1024, NH=16, HD=64.
  qkv = x @ w_qkv ; per-head causal softmax attention ; out = y @ w_proj

Sharding (core c = 4*b + g): data-parallel over batch b (2-way), tensor-
parallel over heads (4-way head-groups g, column-split w_qkv / row-split
w_proj).  Each core computes a full [T, C] partial projection output; the
host sums the 4 partials per batch (the "all-reduce").

Device-side layout trick: the host passes x[b] TRANSPOSED ([C, T]) so the
contraction dim (C) is already on SBUF partitions — no on-device transposes
anywhere.  Attention is computed in the transposed orientation
(S^T = K^T.T @ Q^T with T_k on partitions), so the softmax reduction over
keys comes free from a ones-column appended to V, and A@V needs no
transposes either.

All matmuls run in float32r (fp22 mantissa, 1 PE cycle/row at N>=256).

Program order is staged for cross-engine overlap: Q^T/K^T for head-pair 0
first, V tiles and pair-1 QKV interleaved with pair-0's (ACT-bound)
attention inner loops so the PE always has ready work.
"""

import numpy as np

import concourse.bass as bass
import concourse.tile as tile
import concourse.mybir as mybir
from concourse import bacc
from concourse.bass_utils import run_bass_kernel_spmd

F32 = mybir.dt.float32
F32R = mybir.dt.float32r

B, T, C = 2, 2048, 1024
NH, HD = 16, 64
NCORES = 8
HPC = 4                 # heads per core
WQKV_SL = HPC * HD      # 256 w_qkv columns per section per core
NT = T // 128           # 16 T-chunks of 128
NCC = C // 128          # 8 C-chunks of 128
NG = T // 512           # 4 query groups of 512
MASK_NEG = -1.0e9
DEBUG_DUMPS = False
LOOP_N = 1   # >1: wrap body in an on-device For_i (timing builds)

DBG = {}


def _attention_body(tc):
    nc = tc.nc
    xt_d = nc.dram_tensor("xt", [C, T], F32, kind="ExternalInput")
    wq_d = nc.dram_tensor("wq", [C, WQKV_SL], F32, kind="ExternalInput")
    wk_d = nc.dram_tensor("wk", [C, WQKV_SL], F32, kind="ExternalInput")
    wv_d = nc.dram_tensor("wv", [C, WQKV_SL], F32, kind="ExternalInput")
    wp_d = nc.dram_tensor("wp", [WQKV_SL, C], F32, kind="ExternalInput")
    out_d = nc.dram_tensor("out", [T, C], F32, kind="ExternalOutput")

    Exp = mybir.ActivationFunctionType.Exp

    with (
        tc.tile_pool(name="big", bufs=1) as big,
        tc.tile_pool(name="wts", bufs=1) as wts,
        tc.tile_pool(name="pt", bufs=3) as ptp,
        tc.tile_pool(name="outp", bufs=3) as outp,
        tc.tile_pool(name="norm", bufs=2) as normp,
        tc.tile_pool(name="ps_s", bufs=2, space="PSUM") as ps_s,
        tc.tile_pool(name="ps_acc", bufs=2, space="PSUM") as ps_acc,
        tc.tile_pool(name="ps_ya", bufs=1, space="PSUM") as ps_ya,
        tc.tile_pool(name="ps_yb", bufs=1, space="PSUM") as ps_yb,
    ):
        # ---- constants -------------------------------------------------
        mask_sb = wts.tile([128, 128], F32, tag="mask")
        nc.gpsimd.memset(mask_sb[:, :], 0.0)
        # keep 0 where col >= row (query >= key), else MASK_NEG
        nc.gpsimd.affine_select(
            out=mask_sb[:, :], in_=mask_sb[:, :],
            compare_op=mybir.AluOpType.is_ge,
            fill=MASK_NEG, base=0,
            pattern=[[1, 128]], channel_multiplier=-1,
        )

        loop_ctx = tc.For_i(0, LOOP_N, 1) if LOOP_N > 1 else None
        if loop_ctx is not None:
            loop_ctx.__enter__()

        # ---- input loads (issue order = consumption order) --------------
        wq_sb = wts.tile([128, NCC, WQKV_SL], F32R, tag="wq")
        wk_sb = wts.tile([128, NCC, WQKV_SL], F32R, tag="wk")
        wv_sb = wts.tile([128, NCC, WQKV_SL], F32R, tag="wv")
        wp_sb = wts.tile([128, 2, C], F32R, tag="wp")
        for w_sb, w_d in ((wq_sb, wq_d), (wk_sb, wk_d)):
            nc.sync.dma_start(
                out=w_sb[:, :, :],
                in_=w_d.ap().rearrange("(cc p) n -> p cc n", p=128).bitcast(F32R),
            )
        # x^T in (tg, cc) order so the first Q^T tile's operands land first
        xt_sb = big.tile([128, NCC, T], F32R, tag="xt")
        for tg in range(NG):
            for cc in range(NCC):
                nc.sync.dma_start(
                    out=xt_sb[:, cc, 512 * tg:512 * (tg + 1)],
                    in_=xt_d.ap()[128 * cc:128 * (cc + 1),
                                  512 * tg:512 * (tg + 1)].bitcast(F32R),
                )
        nc.sync.dma_start(
            out=wv_sb[:, :, :],
            in_=wv_d.ap().rearrange("(cc p) n -> p cc n", p=128).bitcast(F32R),
        )
        nc.sync.dma_start(
            out=wp_sb[:, :, :],
            in_=wp_d.ap().rearrange("(k p) n -> p k n", p=128).bitcast(F32R),
        )

        qt = big.tile([128, 2, T], F32R, tag="qt")
        kt = big.tile([128, 2, T], F32R, tag="kt")
        v_sb = big.tile([128, NT, HPC, HD + 1], F32R, tag="v")
        yt = big.tile([128, 2, T], F32R, tag="yt")
        DBG.update(qt=qt.name, kt=kt.name, v=v_sb.name, yt=yt.name)

        ones_sb = wts.tile([128, NT * HPC], F32, tag="ones")
        nc.vector.memset(ones_sb[:, :], 1.0)
        nc.vector.tensor_copy(
            out=v_sb[:, :, :, HD:HD + 1],
            in_=ones_sb[:, :].rearrange("p (a b c) -> p a b c", a=NT, b=HPC),
        )

        def qkt_tiles(k):
            # Q^T / K^T channel tile k (heads 2k, 2k+1), orientation 2
            for w_sb, dst in ((wq_sb, qt), (wk_sb, kt)):
                for tg in range(NG):
                    ps = ps_acc.tile([128, 512], F32, tag="acc")
                    for cc in range(NCC):
                        nc.tensor.matmul(
                            ps[:, :],
                            lhsT=w_sb[:, cc, 128 * k:128 * (k + 1)],
                            rhs=xt_sb[:, cc, 512 * tg:512 * (tg + 1)],
                            start=(cc == 0), stop=(cc == NCC - 1),
                        )
                    nc.vector.tensor_copy(
                        out=dst[:, k, 512 * tg:512 * (tg + 1)], in_=ps[:, :]
                    )

        def v_tiles(t_lo, t_hi):
            # V t-chunks [t_lo, t_hi), orientation 1, into [T, 4, 65] layout
            for ti in range(t_lo, t_hi):
                ps = ps_acc.tile([128, WQKV_SL], F32, tag="acc")
                for cc in range(NCC):
                    nc.tensor.matmul(
                        ps[:, :],
                        lhsT=xt_sb[:, cc, 128 * ti:128 * (ti + 1)],
                        rhs=wv_sb[:, cc, :],
                        start=(cc == 0), stop=(cc == NCC - 1),
                    )
                nc.vector.tensor_copy(
                    out=v_sb[:, ti, :, 0:HD],
                    in_=ps[:, :].rearrange("p (h d) -> p h d", h=HPC),
                )

        def attention_group(pair, g):
            nch = 4 * g + 4
            ya_ps = ps_ya.tile([128, 512], F32, tag="ya")
            yb_ps = ps_yb.tile([128, 512], F32, tag="yb")
            for j in range(nch):
                # S^T for both heads of the pair, row-packed on the PE
                s_ps = ps_s.tile([128, 1024], F32, tag="s")
                pt = ptp.tile([128, 1024], F32R, tag="pt")
                for rowb, colb in ((0, 0), (64, 512)):
                    nc.tensor.matmul(
                        s_ps[:, colb:colb + 512],
                        lhsT=kt[rowb:rowb + 64, pair, 128 * j:128 * (j + 1)],
                        rhs=qt[rowb:rowb + 64, pair, 512 * g:512 * (g + 1)],
                        start=True, stop=True,
                        tile_position=(rowb, 0),
                    )
                if j >= 4 * g:  # diagonal chunk: mask the boundary subtile
                    r = j - 4 * g
                    for colb in (0, 512):
                        cs = colb + 128 * r
                        nc.vector.tensor_add(
                            s_ps[:, cs:cs + 128], s_ps[:, cs:cs + 128],
                            mask_sb[:, :],
                        )
                nc.scalar.activation(
                    out=pt[:, :], in_=s_ps[:, :], func=Exp, scale=1.0 / 8.0,
                )
                # A@V per head (M=65: the ones column lands the softmax
                # denominator in psum row 64), accumulated over j
                c0 = 128 * (j - 4 * g) if j >= 4 * g else 0
                st, sp = (j == 0), (j == nch - 1)
                nc.tensor.matmul(
                    ya_ps[0:65, c0:512],
                    lhsT=v_sb[:, j, 2 * pair, :],
                    rhs=pt[:, c0:512],
                    start=st, stop=sp,
                )
                nc.tensor.matmul(
                    yb_ps[0:65, c0:512],
                    lhsT=v_sb[:, j, 2 * pair + 1, :],
                    rhs=pt[:, 512 + c0:1024],
                    start=st, stop=sp,
                )
            # normalize: yt rows 0-63 = yA/sA, rows 64-127 = yB/sB
            # NB: partition_broadcast reads the tile's physical partition 0
            # (it ignores the AP base partition), so each reciprocal gets its
            # own tile at partition 0.
            recipa_sb = normp.tile([1, 512], F32, tag="recipa")
            recipb_sb = normp.tile([1, 512], F32, tag="recipb")
            bcasta_sb = normp.tile([64, 512], F32, tag="bcasta")
            bcastb_sb = normp.tile([64, 512], F32, tag="bcastb")
            nc.vector.reciprocal(out=recipa_sb[0:1, :], in_=ya_ps[64:65, :])
            nc.vector.reciprocal(out=recipb_sb[0:1, :], in_=yb_ps[64:65, :])
            nc.gpsimd.partition_broadcast(bcasta_sb[:, :], recipa_sb[0:1, :])
            nc.gpsimd.partition_broadcast(bcastb_sb[:, :], recipb_sb[0:1, :])
            gsl = slice(512 * g, 512 * (g + 1))
            nc.vector.tensor_mul(
                yt[0:64, pair, gsl], ya_ps[0:64, :], bcasta_sb[:, :]
            )
            nc.vector.tensor_mul(
                yt[64:128, pair, gsl], yb_ps[0:64, :], bcastb_sb[:, :]
            )

        def proj_block(gb):
            # projection rows 512*gb .. 512*gb+512 (needs yt g-block gb of
            # both pairs)
            for ti in range(4 * gb, 4 * gb + 4):
                for n2 in range(2):
                    ps = ps_acc.tile([128, 512], F32, tag="acc")
                    for k in range(2):
                        nc.tensor.matmul(
                            ps[:, :],
                            lhsT=yt[:, k, 128 * ti:128 * (ti + 1)],
                            rhs=wp_sb[:, k, 512 * n2:512 * (n2 + 1)],
                            start=(k == 0), stop=(k == 1),
                        )
                    o_sb = outp.tile([128, 512], F32, tag="o")
                    nc.vector.tensor_copy(out=o_sb[:, :], in_=ps[:, :])
                    nc.sync.dma_start(
                        out=out_d.ap()[128 * ti:128 * (ti + 1),
                                       512 * n2:512 * (n2 + 1)],
                        in_=o_sb[:, :],
                    )

        # ---- staged schedule -------------------------------------------
        # pair-1 QKV, V tiles and projection blocks are emitted between the
        # (ACT-bound) attention groups so the PE always has ready fill work.
        import os
        sched = os.environ.get("K_SCHED", "C")
        if sched == "A":   # original staging
            qkt_tiles(0)
            v_tiles(0, 4)
            attention_group(0, 0)
            v_tiles(4, 8)
            attention_group(0, 1)
            v_tiles(8, 12)
            qkt_tiles(1)
            attention_group(0, 2)
            v_tiles(12, 16)
            attention_group(0, 3)
            for g in range(NG):
                attention_group(1, g)
            for gb in range(NG):
                proj_block(gb)
        elif sched == "B":  # interleaved pairs + proj blocks
            qkt_tiles(0)
            v_tiles(0, 4)
            attention_group(0, 0)
            qkt_tiles(1)
            attention_group(1, 0)
            proj_block(0)
            v_tiles(4, 8)
            attention_group(0, 1)
            attention_group(1, 1)
            proj_block(1)
            v_tiles(8, 12)
            attention_group(0, 2)
            attention_group(1, 2)
            proj_block(2)
            v_tiles(12, 16)
            attention_group(0, 3)
            attention_group(1, 3)
            proj_block(3)
        elif sched == "C":  # interleaved pairs, proj one block behind
            qkt_tiles(0)
            v_tiles(0, 4)
            attention_group(0, 0)
            qkt_tiles(1)
            attention_group(1, 0)
            v_tiles(4, 8)
            attention_group(0, 1)
            proj_block(0)
            attention_group(1, 1)
            v_tiles(8, 12)
            attention_group(0, 2)
            proj_block(1)
            attention_group(1, 2)
            v_tiles(12, 16)
            attention_group(0, 3)
            proj_block(2)
            attention_group(1, 3)
            proj_block(3)

        if loop_ctx is not None:
            loop_ctx.__exit__(None, None, None)

        if DEBUG_DUMPS:
            for nm, tl in (("d_qt", qt), ("d_kt", kt), ("d_yt", yt)):
                dd = nc.dram_tensor(nm, [128, 2 * T], F32, kind="ExternalOutput")
                nc.sync.dma_start(out=dd.ap(), in_=tl[:, :, :].bitcast(F32))
            dv = nc.dram_tensor("d_v", [128, NT * HPC * (HD + 1)], F32,
                                kind="ExternalOutput")
            nc.sync.dma_start(out=dv.ap(), in_=v_sb[:, :, :, :].bitcast(F32))


_NC_CACHE = None


def build_nc():
    global _NC_CACHE
    if _NC_CACHE is not None:
        return _NC_CACHE
    nc = bacc.Bacc("TRN2", target_bir_lowering=False, debug=False,
                   num_devices=NCORES)
    with tile.TileContext(nc) as tc:
        _attention_body(tc)
    nc.compile()
    _NC_CACHE = nc
    return nc


def make_in_maps(x, w_qkv, w_proj):
    x = np.asarray(x, dtype=np.float32)
    w_qkv = np.asarray(w_qkv, dtype=np.float32)
    w_proj = np.asarray(w_proj, dtype=np.float32)
    in_maps = []
    for c in range(NCORES):
        b, g = divmod(c, NCORES // B)
        sl = slice(WQKV_SL * g, WQKV_SL * (g + 1))
        in_maps.append({
            "xt": np.ascontiguousarray(x[b].T),
            "wq": np.ascontiguousarray(w_qkv[:, sl]),
            "wk": np.ascontiguousarray(w_qkv[:, C:][:, sl]),
            "wv": np.ascontiguousarray(w_qkv[:, 2 * C:][:, sl]),
            "wp": np.ascontiguousarray(w_proj[sl, :]),
        })
    return in_maps


def kernel(x, w_qkv, w_proj):
    nc = build_nc()
    in_maps = make_in_maps(x, w_qkv, w_proj)
    res = run_bass_kernel_spmd(nc, in_maps, core_ids=list(range(NCORES)))
    out = np.empty((B, T, C), dtype=np.float32)
    npc = NCORES // B
    for b in range(B):
        acc = res.results[npc * b]["out"].astype(np.float32)
        for g in range(1, npc):
            acc = acc + res.results[npc * b + g]["out"]
        out[b] = acc
    return out
